# revision 19
# baseline (speedup 1.0000x reference)
"""DTNN layer kernel for Trainium2 (8 NeuronCores).

Math: out[b,i,o] = sum_j sum_h Wfc[o,h] * hx[b,i,h] * hd[b,i,j,h]
with hx = x@Wcf.T + bcf, hd = dist@Wdf.T + bdf.
Since Wfc/Wdf are linear, the j-sum commutes:
    ds[b,i,d]  = sum_j dist[b,i,j,d]                  (memory-bound reduction)
    out[b,i,:] = ((x@Wcf.T + bcf) * (ds@Wdf.T + N*bdf)) @ Wfc.T
So the kernel streams `distance` once (134MB) and does a few 128x128 matmuls.

Sharding: flatten (B,N) -> 1024 i-rows, 128 rows per core; no cross-core comms.

v2 design (from NTFF trace analysis of v1, which ran ~70-75us):
- v1's fold (DVE halving adds, ~41us busy) lagged the 43.5us DMA stream by
  ~19us because the big 64-j tiles were folded big-first (DVE idle until the
  first 4.2MB tile landed at ~17us) and the j=4 endgame tiles trickled at
  ~100GB/s (2KB per-partition lines expose HBM latency). Serial tail ~5us.
- v2: mostly-32j tiles (16KB lines, full-rate packets) tapering to 8j so the
  fold tracks arrivals; fold mid-stages run in bf16 (2x DVE rate), stage 1
  casts fp32->bf16, last stage emits a 128-col fp32 chunk result.
- Each chunk result is transposed on the (idle) PE with an accumulating
  is_transpose matmul into one PSUM tile: ds^T accumulation is free, no DVE
  acc-adds and no post-stream transpose.
- All PE matmuls run bf16 (1 cycle/row vs 4 for fp32): weights/x/biases are
  pre-cast host-side. Output is computed mirrored (out^T = WfcT^T @ sT) so
  the final matmul's stationary operand is a constant; host transposes back.
- PSUM->SBUF copies ride the Scalar(ACT) engine, keeping DVE for folds only.
Numpy-simulated rel err of this scheme: 4.1e-3 (gate 2e-2).
"""

import numpy as np
from ml_dtypes import bfloat16

import concourse.bass as bass
import concourse.bacc as bacc
import concourse.mybir as mybir
from concourse.tile import TileContext
from concourse.bass_utils import run_bass_kernel_spmd

B, N, D, H = 4, 256, 128, 128
NCORES = 8
ROWS = B * N // NCORES  # 128 i-rows per core
FP = mybir.dt.float32
BF = mybir.dt.bfloat16

# j-counts per streamed tile: 32j tiles (16KB per-partition lines = full-rate
# 16KB DMA packets) tapering to 8j so the last fold chain is short.
SIZES = [16, 32, 32, 32, 32, 32, 32, 16, 8, 8, 8, 8]
assert sum(SIZES) == N

# bf16 constant block columns: [xT | wcfT | wdfT | wfcT | bcf_row | ones_row]
CB_XT = 0
CB_WCF = 128
CB_WDF = 256
CB_WFC = 384
CB_BCFR = 512   # partition 0: bcf row (1, H)
CB_ONES = 640   # partition 0: ones row (1, ROWS)
CB_TOT = 768

# fp32 constant block columns: [eye | N*bdf col]
CF_EYE = 0
CF_BDFN = 128   # per-partition column (H, 1) = N * bdf
CF_TOT = 129


def build_nc():
    nc = bacc.Bacc("TRN2", target_bir_lowering=False)
    dist = nc.declare_dram_parameter("dist", [ROWS, N * D], FP, isOutput=False)
    cstb = nc.declare_dram_parameter("cstb", [128, CB_TOT], BF, isOutput=False)
    cstf = nc.declare_dram_parameter("cstf", [128, CF_TOT], FP, isOutput=False)
    out = nc.declare_dram_parameter("out", [2 * D, ROWS // 2], FP,
                                    isOutput=True)

    with TileContext(nc) as tc:
        with (
            tc.tile_pool(name="const", bufs=1) as cpool,
            tc.tile_pool(name="dist", bufs=1) as dpool,
            tc.tile_pool(name="scratch", bufs=1) as spool,
            tc.tile_pool(name="work", bufs=1) as wpool,
            tc.tile_pool(name="psum", bufs=1, space="PSUM") as ppool,
        ):
            # dist stream first so the big DMAs start ASAP (sync HWDGE ring);
            # constants ride the scalar HWDGE ring concurrently.
            dtiles = []
            off = 0
            for k, jn in enumerate(SIZES):
                t = dpool.tile([ROWS, jn * D], FP, tag=f"dist{k}")
                nc.sync.dma_start(out=t[:], in_=dist[:, off * D:(off + jn) * D])
                dtiles.append(t)
                off += jn

            cstb_t = cpool.tile([128, CB_TOT], BF, tag="cstb")
            nc.scalar.dma_start(out=cstb_t[:], in_=cstb[:])
            cstf_t = cpool.tile([128, CF_TOT], FP, tag="cstf")
            nc.scalar.dma_start(out=cstf_t[:], in_=cstf[:])
            xT_t = cstb_t[:, CB_XT:CB_XT + ROWS]
            wcf_t = cstb_t[:, CB_WCF:CB_WCF + H]
            wdf_t = cstb_t[:, CB_WDF:CB_WDF + H]
            wfc_t = cstb_t[:, CB_WFC:CB_WFC + D]
            bcf_row = cstb_t[0:1, CB_BCFR:CB_BCFR + H]
            ones_row = cstb_t[0:1, CB_ONES:CB_ONES + ROWS]
            ident = cstf_t[:, CF_EYE:CF_EYE + ROWS]
            bdfN = cstf_t[:, CF_BDFN:CF_BDFN + 1]

            # hx^T = Wcf @ x^T + bcf x ones -> (H, ROWS) in PSUM (bf16 mms)
            hx_ps = ppool.tile([H, ROWS], FP, tag="hx_ps")
            nc.tensor.matmul(hx_ps[:], wcf_t, xT_t, start=True, stop=False)
            nc.tensor.matmul(hx_ps[:], bcf_row, ones_row, start=False, stop=True)
            # fp32 copy for the final DVE mul; bf16 scaled copy for the bias
            # preload term. Both on ACT, reading hx straight from PSUM.
            hxT = wpool.tile([H, ROWS], FP, tag="hxT")
            nc.scalar.copy(hxT[:], hx_ps[:])
            s0T = wpool.tile([H, ROWS], BF, tag="s0T")
            nc.scalar.mul(s0T[:], hx_ps[:], bdfN)

            # Preload the bias term (hx * N*bdf) @ Wfc^T (mirrored: into
            # out^T PSUM); the final matmul accumulates onto it.
            outT_ps = ppool.tile([D, ROWS], FP, tag="outT_ps")
            nc.tensor.matmul(outT_ps[:], wfc_t, s0T[:], start=True, stop=False,
                             skip_group_check=True)

            # Streaming j-reduction: each tile halved with DVE adds (stage 1
            # casts fp32->bf16, mid stages run bf16 at 2x rate, last stage
            # emits fp32 128 cols), then the chunk result is transposed on
            # the PE, accumulating ds^T in PSUM across chunks.
            dsT_ps = ppool.tile([D, ROWS], FP, tag="dsT_ps")
            scs = [spool.tile([ROWS, 64 * max(SIZES)], BF, name=f"sc{i}",
                              tag=f"sc{i}") for i in range(2)]
            rts = [wpool.tile([ROWS, D], FP, name=f"r{i}", tag=f"r{i}")
                   for i in range(2)]
            with nc.allow_low_precision("fold mid-stages in bf16; validated "
                                        "rel err 4e-3 vs 2e-2 budget"):
                for k, jn in enumerate(SIZES):
                    t, sc, r = dtiles[k], scs[k % 2], rts[k % 2]
                    half = jn * D // 2
                    nc.vector.tensor_add(
                        sc[:, 0:half], t[:, 0:half], t[:, half:2 * half]
                    )
                    c = half // 2
                    while c > D:
                        nc.vector.tensor_add(
                            sc[:, 0:c], sc[:, 0:c], sc[:, c:2 * c]
                        )
                        c //= 2
                    nc.vector.tensor_add(r[:], sc[:, 0:D], sc[:, D:2 * D])
                    nc.tensor.matmul(dsT_ps[:], r[:], ident,
                                     is_transpose=True, start=(k == 0),
                                     stop=(k == len(SIZES) - 1),
                                     skip_group_check=True)

            # Tail: dsT -> bf16, hd^T = Wdf @ ds^T, s^T = hx^T * hd^T,
            # out^T += Wfc @ s^T (onto preloaded bias term), store.
            dsT_b = wpool.tile([D, ROWS], BF, tag="dsT_b")
            hd_ps = ppool.tile([H, ROWS], FP, tag="hd_ps")
            sT = wpool.tile([H, ROWS], BF, tag="sT")
            out_sb = wpool.tile([D, ROWS], FP, tag="out_sb")
            HALVES = ((0, 64), (64, 128))
            for h0, h1 in HALVES:
                nc.scalar.copy(dsT_b[:, h0:h1], dsT_ps[:, h0:h1])
            for h0, h1 in HALVES:
                nc.tensor.matmul(hd_ps[:, h0:h1], wdf_t, dsT_b[:, h0:h1],
                                 start=True, stop=True, skip_group_check=True)
            for h0, h1 in HALVES:
                nc.vector.tensor_mul(sT[:, h0:h1], hd_ps[:, h0:h1],
                                     hxT[:, h0:h1])
            for h0, h1 in HALVES:
                nc.tensor.matmul(outT_ps[:, h0:h1], wfc_t, sT[:, h0:h1],
                                 start=False, stop=(h0 == 64),
                                 skip_group_check=True)
            for hi, (h0, h1) in enumerate(HALVES):
                nc.scalar.copy(out_sb[:, h0:h1], outT_ps[:, h0:h1])
                nc.scalar.dma_start(out=out[hi * D:(hi + 1) * D, :],
                                    in_=out_sb[:, h0:h1])
    nc.compile()
    return nc


_NC_CACHE = None


def _get_nc():
    global _NC_CACHE
    if _NC_CACHE is None:
        _NC_CACHE = build_nc()
    return _NC_CACHE


def _make_in_maps(x, distance, Wcf_w, Wcf_b, Wdf_w, Wdf_b, Wfc_w):
    x = np.ascontiguousarray(np.asarray(x, np.float32))
    distance = np.ascontiguousarray(np.asarray(distance, np.float32))
    x_flat = x.reshape(B * N, D)
    dist_flat = distance.reshape(B * N, N * D)
    wcfT = np.asarray(Wcf_w, np.float32).T.astype(bfloat16)
    wdfT = np.asarray(Wdf_w, np.float32).T.astype(bfloat16)
    wfcT = np.asarray(Wfc_w, np.float32).T.astype(bfloat16)
    bcf = np.asarray(Wcf_b, np.float32).astype(bfloat16)
    bdfN = (np.asarray(Wdf_b, np.float32) * float(N))
    cstf_blk = np.zeros((128, CF_TOT), np.float32)
    cstf_blk[:, CF_EYE:CF_EYE + ROWS] = np.eye(ROWS, dtype=np.float32)
    cstf_blk[:, CF_BDFN] = bdfN
    in_maps = []
    for c in range(NCORES):
        sl = slice(c * ROWS, (c + 1) * ROWS)
        cstb_blk = np.zeros((128, CB_TOT), bfloat16)
        cstb_blk[:, CB_XT:CB_XT + ROWS] = x_flat[sl].T.astype(bfloat16)
        cstb_blk[:, CB_WCF:CB_WCF + H] = wcfT
        cstb_blk[:, CB_WDF:CB_WDF + H] = wdfT
        cstb_blk[:, CB_WFC:CB_WFC + D] = wfcT
        cstb_blk[0, CB_BCFR:CB_BCFR + H] = bcf
        cstb_blk[0, CB_ONES:CB_ONES + ROWS] = bfloat16(1.0)
        in_maps.append({
            "dist": np.ascontiguousarray(dist_flat[sl]),
            "cstb": cstb_blk,
            "cstf": cstf_blk,
        })
    return in_maps


def kernel(x, distance, Wcf_w, Wcf_b, Wdf_w, Wdf_b, Wfc_w):
    in_maps = _make_in_maps(x, distance, Wcf_w, Wcf_b, Wdf_w, Wdf_b, Wfc_w)
    nc = _get_nc()
    res = run_bass_kernel_spmd(nc, in_maps, list(range(NCORES))).results
    # per-core result is out^T stored as two stacked [D, 64] i-halves
    outs = []
    for c in range(NCORES):
        a = res[c]["out"]  # [2*D, 64]
        outT = np.concatenate([a[0:D, :], a[D:2 * D, :]], axis=1)
        outs.append(np.ascontiguousarray(outT.T))
    return np.concatenate(outs, axis=0).reshape(B, N, D)
